# revision 1
# baseline (speedup 1.0000x reference)
"""Causal multi-head attention (b=2, n=2048, d=768, 12 heads) on 8 TRN2 NeuronCores.

Sharding: batch x head-group. Core c handles batch c//4 and heads 3*(c%4) .. 3*(c%4)+2.
Each core gets xT = x[b].T plus the W.T column slices for its 3 heads, computes its
[2048, 192] output slab; the host concatenates slabs into the full [2, 2048, 768].

Per-core algorithm (everything transposed so softmax reductions ride on matmuls):
  qT/kT/vT[h] = (W.T slice).T @ xT          TensorE, per 512-col span
  v_nat[j, m] = transpose(vT) + ones column -> stationary [128, 65] per j-tile
  per head, per 512-row i-span:
    sT[j, i] = kT[:, jtile].T @ qT[:, span]  (psum, causally skipped tiles)
    p = exp(sT)  unshifted (max causal score ~66 fits fp32); diagonal tile
        gets zero-fill + triangular mask
    av[0:65, span] += v_nat[jtile].T @ p     (row 64 accumulates sum(p) = softmax denom)
  finalize: transpose av back per 128 cols, multiply rows by 1/denom, DMA out.
"""
import sys

if "/opt/trn_rl_repo" not in sys.path:
    sys.path.insert(0, "/opt/trn_rl_repo")

from contextlib import ExitStack

import numpy as np

import concourse.bass as bass
import concourse.tile as tile
from concourse import bacc, mybir, bass_utils
from concourse.masks import make_identity, make_upper_triangular

F32 = mybir.dt.float32
F32R = mybir.dt.float32r

P = 128
SPAN = 512
HD = 64

# problem constants
B, N, D, NH = 2, 2048, 768, 12
HL = 3                      # heads per core
DL = HL * HD                # 192
N_CORES = 8

# dtype config (f32r = fast reduced-precision PE mode, ~1e-4/element)
DT_PROJ = F32R
DT_SC = F32R
DT_AV = F32R


def _build(nc, *, n, d, hl, dt_proj, dt_sc, dt_av):
    KT = d // P
    NS = n // SPAN
    NT = n // P
    CPS = SPAN // P
    dl = hl * HD
    n_mt = (hl + 1) // 2

    xt = nc.dram_tensor("xt", [d, n], dt_proj, kind="ExternalInput").ap()
    wq = nc.dram_tensor("wq", [d, dl], dt_proj, kind="ExternalInput").ap()
    wk = nc.dram_tensor("wk", [d, dl], dt_proj, kind="ExternalInput").ap()
    wv = nc.dram_tensor("wv", [d, dl], dt_proj, kind="ExternalInput").ap()
    o = nc.dram_tensor("o", [n, dl], F32, kind="ExternalOutput").ap()

    m_chunks = []
    off = 0
    while off < dl:
        sz = min(P, dl - off)
        m_chunks.append((off, sz))
        off += sz

    def head_slice(tiles, h, c0, c1):
        t = tiles[h // 2]
        b = (h % 2) * HD
        return t[b:b + HD, c0:c1]

    tc = nc._tc
    with ExitStack() as ctx:
        const_pool = ctx.enter_context(tc.tile_pool(name="const", bufs=1))
        xpool = ctx.enter_context(tc.tile_pool(name="x", bufs=KT * NS))
        wpool = ctx.enter_context(tc.tile_pool(name="w", bufs=3 * KT))
        qkv_pool = ctx.enter_context(tc.tile_pool(name="qkv", bufs=3 * 2 * NS))
        vnat_pool = ctx.enter_context(tc.tile_pool(name="vnat", bufs=hl))
        ppool = ctx.enter_context(tc.tile_pool(name="p", bufs=3))
        avt_pool = ctx.enter_context(tc.tile_pool(name="avt", bufs=2))
        osb_pool = ctx.enter_context(tc.tile_pool(name="osb", bufs=3))
        ps_proj = ctx.enter_context(tc.tile_pool(name="ps_proj", bufs=2, space="PSUM"))
        ps_vtr = ctx.enter_context(tc.tile_pool(name="ps_vtr", bufs=1, space="PSUM"))
        ps_sc = ctx.enter_context(tc.tile_pool(name="ps_sc", bufs=2, space="PSUM"))
        ps_av = ctx.enter_context(tc.tile_pool(name="ps_av", bufs=2, space="PSUM"))
        ps_otr = ctx.enter_context(tc.tile_pool(name="ps_otr", bufs=1, space="PSUM"))

        ident = const_pool.tile([P, P], F32)
        make_identity(nc, ident[:])
        trimask = const_pool.tile([P, P], F32)
        make_upper_triangular(nc, trimask[:], val=1.0, diag=True)
        ones16 = const_pool.tile([P, NT], F32)
        nc.gpsimd.memset(ones16[:], 1.0)
        zeros384 = const_pool.tile([P, SPAN - P], F32)
        nc.gpsimd.memset(zeros384[:], 0.0)

        w_tiles = {}
        for name, wsrc in (("q", wq), ("k", wk), ("v", wv)):
            lst = []
            for kt in range(KT):
                t = wpool.tile([P, dl], dt_proj, tag="w")
                nc.sync.dma_start(t[:], wsrc[kt * P:(kt + 1) * P, :])
                lst.append(t)
            w_tiles[name] = lst

        x_tiles = {}
        for ns in range(NS):
            for kt in range(KT):
                t = xpool.tile([P, SPAN], dt_proj, tag="x")
                nc.sync.dma_start(
                    t[:], xt[kt * P:(kt + 1) * P, ns * SPAN:(ns + 1) * SPAN])
                x_tiles[(kt, ns)] = t

        qkvT = {"q": [], "k": [], "v": []}
        for name in ("q", "k", "v"):
            for mt, (moff, msz) in enumerate(m_chunks):
                qkvT[name].append([])
                for ns in range(NS):
                    pt = ps_proj.tile([msz, SPAN], F32, tag="ps_proj")
                    for kt in range(KT):
                        nc.tensor.matmul(
                            pt[:],
                            w_tiles[name][kt][:, moff:moff + msz],
                            x_tiles[(kt, ns)][:],
                            start=(kt == 0), stop=(kt == KT - 1))
                    st = qkv_pool.tile(
                        [msz, SPAN],
                        dt_sc if name in ("q", "k") else F32, tag="qkv")
                    nc.vector.tensor_copy(st[:], pt[:])
                    qkvT[name][mt].append(st)

        v_nat = []
        for h in range(hl):
            vt = vnat_pool.tile([P, NT * (HD + 1)], dt_av, tag="vnat")
            v3 = vt[:].rearrange("p (t c) -> p t c", c=HD + 1)
            nc.vector.tensor_copy(v3[:, :, HD], ones16[:])
            for jt in range(NT):
                ns, c = jt // CPS, jt % CPS
                src = head_slice([qkvT["v"][i][ns] for i in range(n_mt)],
                                 h, c * P, (c + 1) * P)
                pt = ps_vtr.tile([P, HD], F32, tag="ps_vtr")
                b = (h % 2) * HD
                nc.tensor.transpose(pt[:], src, ident[b:b + HD, b:b + HD])
                nc.vector.tensor_copy(vt[:, jt * (HD + 1):jt * (HD + 1) + HD],
                                      pt[:])
            v_nat.append(vt)

        for h in range(hl):
            for s in range(NS):
                av = ps_av.tile([HD + 1, SPAN], F32, tag="ps_av")
                njt = CPS * s + CPS
                for jt in range(njt):
                    c_d = jt - CPS * s
                    n0 = max(c_d, 0) * P
                    ns_k, ck = jt // CPS, jt % CPS
                    sc = ps_sc.tile([P, SPAN], F32, tag="ps_sc")
                    nc.tensor.matmul(
                        sc[:, n0:SPAN],
                        head_slice([qkvT["k"][i][ns_k] for i in range(n_mt)],
                                   h, ck * P, (ck + 1) * P),
                        head_slice([qkvT["q"][i][s] for i in range(n_mt)],
                                   h, n0, SPAN),
                        start=True, stop=True)
                    p = ppool.tile([P, SPAN], dt_av, tag="p")
                    if n0 > 0:
                        if dt_av == F32:
                            nc.gpsimd.memset(p[:, 0:n0], 0.0)
                        else:
                            nc.vector.tensor_copy(p[:, 0:n0], zeros384[:, 0:n0])
                    nc.scalar.activation(p[:, n0:SPAN], sc[:, n0:SPAN],
                                         mybir.ActivationFunctionType.Exp)
                    if c_d >= 0:
                        nc.vector.tensor_mul(
                            p[:, c_d * P:(c_d + 1) * P],
                            p[:, c_d * P:(c_d + 1) * P],
                            trimask[:])
                    nc.tensor.matmul(
                        av[:],
                        v_nat[h][:, jt * (HD + 1):(jt + 1) * (HD + 1)],
                        p[:],
                        start=(jt == 0), stop=(jt == njt - 1))
                avt = avt_pool.tile([HD + 1, SPAN], F32, tag="avt")
                nc.vector.tensor_copy(avt[:], av[:])
                for c in range(CPS):
                    pt = ps_otr.tile([P, HD + 1], F32, tag="ps_otr")
                    nc.tensor.transpose(pt[:], avt[:, c * P:(c + 1) * P],
                                        ident[0:HD + 1, 0:HD + 1])
                    ob = osb_pool.tile([P, HD], F32, tag="osb")
                    rcp = osb_pool.tile([P, 1], F32, tag="rcp")
                    nc.vector.reciprocal(rcp[:], pt[:, HD:HD + 1])
                    nc.vector.tensor_scalar_mul(ob[:], pt[:, 0:HD], rcp[:])
                    i0 = (s * CPS + c) * P
                    nc.sync.dma_start(o[i0:i0 + P, h * HD:(h + 1) * HD], ob[:])


_NC_CACHE = {}


def _get_module(dt_proj=DT_PROJ, dt_sc=DT_SC, dt_av=DT_AV):
    key = (dt_proj, dt_sc, dt_av)
    if key not in _NC_CACHE:
        nc = bacc.Bacc("TRN2", target_bir_lowering=False, debug=False)
        with tile.TileContext(nc) as tc:
            nc._tc = tc
            _build(nc, n=N, d=D, hl=HL,
                   dt_proj=dt_proj, dt_sc=dt_sc, dt_av=dt_av)
        nc.compile()
        _NC_CACHE[key] = nc
    return _NC_CACHE[key]


def _in_maps(x, Wq, Wk, Wv):
    maps = []
    xT = [np.ascontiguousarray(x[b].T) for b in range(B)]
    WqT, WkT, WvT = Wq.T, Wk.T, Wv.T
    for c in range(N_CORES):
        bc, g = divmod(c, N_CORES // B)
        sl = slice(g * DL, (g + 1) * DL)
        maps.append({
            "xt": xT[bc],
            "wq": np.ascontiguousarray(WqT[:, sl]),
            "wk": np.ascontiguousarray(WkT[:, sl]),
            "wv": np.ascontiguousarray(WvT[:, sl]),
        })
    return maps


def kernel(x, Wq, Wk, Wv, _trace=False, **_kw):
    x = np.asarray(x, dtype=np.float32)
    Wq = np.asarray(Wq, dtype=np.float32)
    Wk = np.asarray(Wk, dtype=np.float32)
    Wv = np.asarray(Wv, dtype=np.float32)
    assert x.shape == (B, N, D) and Wq.shape == (D, D)

    nc = _get_module()
    res = bass_utils.run_bass_kernel_spmd(
        nc, _in_maps(x, Wq, Wk, Wv), core_ids=list(range(N_CORES)),
        trace=_trace)
    out = np.empty((B, N, D), np.float32)
    for c in range(N_CORES):
        bc, g = divmod(c, N_CORES // B)
        out[bc, :, g * DL:(g + 1) * DL] = res.results[c]["o"]
    if _trace:
        return out, res
    return out


# revision 3
# speedup vs baseline: 1.0025x; 1.0025x over previous
"""Causal multi-head attention (b=2, n=2048, d=768, 12 heads) on 8 TRN2 NeuronCores.

Sharding: batch x head-group. Core c handles batch c//4 and heads 3*(c%4) .. 3*(c%4)+2.
Each core gets xT = x[b].T plus the W.T column slices for its 3 heads, computes its
[2048, 192] output slab; the host concatenates slabs into the full [2, 2048, 768].

Per-core algorithm (everything transposed so softmax reductions ride on matmuls):
  qT/kT/vT[h] = (W.T slice).T @ xT          TensorE, per 512-col span
  v_nat[j, m] = transpose(vT) + ones column -> stationary [128, 65] per j-tile
  per head, per 512-row i-span:
    sT[j, i] = kT[:, jtile].T @ qT[:, span]  (psum, causally skipped tiles)
    p = exp(sT)  unshifted (max causal score ~66 fits fp32); diagonal tile
        gets zero-fill + triangular mask
    av[0:65, span] += v_nat[jtile].T @ p     (row 64 accumulates sum(p) = softmax denom)
  finalize: transpose av back per 128 cols, multiply rows by 1/denom, DMA out.
"""
import sys

if "/opt/trn_rl_repo" not in sys.path:
    sys.path.insert(0, "/opt/trn_rl_repo")

from contextlib import ExitStack

import numpy as np

import concourse.bass as bass
import concourse.tile as tile
from concourse import bacc, mybir, bass_utils
from concourse.masks import make_identity, make_upper_triangular

F32 = mybir.dt.float32
F32R = mybir.dt.float32r

P = 128
SPAN = 512
HD = 64

# problem constants
B, N, D, NH = 2, 2048, 768, 12
HL = 3                      # heads per core
DL = HL * HD                # 192
N_CORES = 8

# dtype config (f32r = fast reduced-precision PE mode, ~1e-4/element)
DT_PROJ = F32R
DT_SC = F32R
DT_AV = F32R


def _build(nc, *, n, d, hl, dt_proj, dt_sc, dt_av):
    KT = d // P
    NS = n // SPAN
    NT = n // P
    CPS = SPAN // P
    dl = hl * HD
    n_mt = (hl + 1) // 2

    xt = nc.dram_tensor("xt", [d, n], dt_proj, kind="ExternalInput").ap()
    wq = nc.dram_tensor("wq", [d, dl], dt_proj, kind="ExternalInput").ap()
    wk = nc.dram_tensor("wk", [d, dl], dt_proj, kind="ExternalInput").ap()
    wv = nc.dram_tensor("wv", [d, dl], dt_proj, kind="ExternalInput").ap()
    o = nc.dram_tensor("o", [n, dl], F32, kind="ExternalOutput").ap()

    m_chunks = []
    off = 0
    while off < dl:
        sz = min(P, dl - off)
        m_chunks.append((off, sz))
        off += sz

    def head_slice(tiles, h, c0, c1):
        t = tiles[h // 2]
        b = (h % 2) * HD
        return t[b:b + HD, c0:c1]

    tc = nc._tc
    with ExitStack() as ctx:
        const_pool = ctx.enter_context(tc.tile_pool(name="const", bufs=1))
        xpool = ctx.enter_context(tc.tile_pool(name="x", bufs=KT * NS))
        wpool = ctx.enter_context(tc.tile_pool(name="w", bufs=3 * KT))
        qkv_pool = ctx.enter_context(tc.tile_pool(name="qkv", bufs=3 * 2 * NS))
        vnat_pool = ctx.enter_context(tc.tile_pool(name="vnat", bufs=hl))
        ppool = ctx.enter_context(tc.tile_pool(name="p", bufs=3))
        avt_pool = ctx.enter_context(tc.tile_pool(name="avt", bufs=2))
        osb_pool = ctx.enter_context(tc.tile_pool(name="osb", bufs=3))
        ps_proj = ctx.enter_context(tc.tile_pool(name="ps_proj", bufs=2, space="PSUM"))
        ps_vtr = ctx.enter_context(tc.tile_pool(name="ps_vtr", bufs=1, space="PSUM"))
        ps_sc = ctx.enter_context(tc.tile_pool(name="ps_sc", bufs=2, space="PSUM"))
        ps_av = ctx.enter_context(tc.tile_pool(name="ps_av", bufs=2, space="PSUM"))
        ps_otr = ctx.enter_context(tc.tile_pool(name="ps_otr", bufs=1, space="PSUM"))

        ident = const_pool.tile([P, P], F32)
        make_identity(nc, ident[:])
        trimask = const_pool.tile([P, P], F32)
        make_upper_triangular(nc, trimask[:], val=1.0, diag=True)
        ones16 = const_pool.tile([P, NT], F32)
        nc.gpsimd.memset(ones16[:], 1.0)
        zeros384 = const_pool.tile([P, SPAN - P], F32)
        nc.gpsimd.memset(zeros384[:], 0.0)

        w_tiles = {}
        for name, wsrc in (("q", wq), ("k", wk), ("v", wv)):
            lst = []
            for kt in range(KT):
                t = wpool.tile([P, dl], dt_proj, tag="w")
                nc.sync.dma_start(t[:], wsrc[kt * P:(kt + 1) * P, :])
                lst.append(t)
            w_tiles[name] = lst

        x_tiles = {}
        for ns in range(NS):
            for kt in range(KT):
                t = xpool.tile([P, SPAN], dt_proj, tag="x")
                nc.sync.dma_start(
                    t[:], xt[kt * P:(kt + 1) * P, ns * SPAN:(ns + 1) * SPAN])
                x_tiles[(kt, ns)] = t

        qkvT = {"q": [], "k": [], "v": []}
        for name in ("q", "k", "v"):
            for mt, (moff, msz) in enumerate(m_chunks):
                qkvT[name].append([])
                for ns in range(NS):
                    pt = ps_proj.tile([msz, SPAN], F32, tag="ps_proj")
                    for kt in range(KT):
                        nc.tensor.matmul(
                            pt[:],
                            w_tiles[name][kt][:, moff:moff + msz],
                            x_tiles[(kt, ns)][:],
                            start=(kt == 0), stop=(kt == KT - 1))
                    st = qkv_pool.tile(
                        [msz, SPAN],
                        dt_sc if name in ("q", "k") else F32, tag="qkv")
                    nc.vector.tensor_copy(st[:], pt[:])
                    qkvT[name][mt].append(st)

        v_nat = []
        for h in range(hl):
            vt = vnat_pool.tile([P, NT * (HD + 1)], dt_av, tag="vnat")
            v3 = vt[:].rearrange("p (t c) -> p t c", c=HD + 1)
            nc.vector.tensor_copy(v3[:, :, HD], ones16[:])
            for jt in range(NT):
                ns, c = jt // CPS, jt % CPS
                src = head_slice([qkvT["v"][i][ns] for i in range(n_mt)],
                                 h, c * P, (c + 1) * P)
                pt = ps_vtr.tile([P, HD], F32, tag="ps_vtr")
                b = (h % 2) * HD
                nc.tensor.transpose(pt[:], src, ident[b:b + HD, b:b + HD])
                nc.vector.tensor_copy(vt[:, jt * (HD + 1):jt * (HD + 1) + HD],
                                      pt[:])
            v_nat.append(vt)

        for h in range(hl):
            for s in range(NS):
                av = ps_av.tile([HD + 1, SPAN], F32, tag="ps_av")
                njt = CPS * s + CPS
                for jt in range(njt):
                    c_d = jt - CPS * s
                    n0 = max(c_d, 0) * P
                    ns_k, ck = jt // CPS, jt % CPS
                    sc = ps_sc.tile([P, SPAN], F32, tag="ps_sc")
                    nc.tensor.matmul(
                        sc[:, n0:SPAN],
                        head_slice([qkvT["k"][i][ns_k] for i in range(n_mt)],
                                   h, ck * P, (ck + 1) * P),
                        head_slice([qkvT["q"][i][s] for i in range(n_mt)],
                                   h, n0, SPAN),
                        start=True, stop=True)
                    p = ppool.tile([P, SPAN], dt_av, tag="p")
                    if n0 > 0:
                        if dt_av == F32:
                            nc.gpsimd.memset(p[:, 0:n0], 0.0)
                        else:
                            nc.vector.tensor_copy(p[:, 0:n0], zeros384[:, 0:n0])
                    nc.scalar.activation(p[:, n0:SPAN], sc[:, n0:SPAN],
                                         mybir.ActivationFunctionType.Exp)
                    if c_d >= 0:
                        nc.vector.tensor_mul(
                            p[:, c_d * P:(c_d + 1) * P],
                            p[:, c_d * P:(c_d + 1) * P],
                            trimask[:])
                    nc.tensor.matmul(
                        av[:],
                        v_nat[h][:, jt * (HD + 1):(jt + 1) * (HD + 1)],
                        p[:],
                        start=(jt == 0), stop=(jt == njt - 1))
                avt = avt_pool.tile([HD + 1, SPAN], F32, tag="avt")
                nc.vector.tensor_copy(avt[:], av[:])
                for c in range(CPS):
                    pt = ps_otr.tile([P, HD + 1], F32, tag="ps_otr")
                    nc.tensor.transpose(pt[:], avt[:, c * P:(c + 1) * P],
                                        ident[0:HD + 1, 0:HD + 1])
                    ob = osb_pool.tile([P, HD], F32, tag="osb")
                    rcp = osb_pool.tile([P, 1], F32, tag="rcp")
                    nc.vector.reciprocal(rcp[:], pt[:, HD:HD + 1])
                    nc.vector.tensor_scalar_mul(ob[:], pt[:, 0:HD], rcp[:])
                    i0 = (s * CPS + c) * P
                    nc.sync.dma_start(o[i0:i0 + P, h * HD:(h + 1) * HD], ob[:])


_NC_CACHE = {}


def _get_module(dt_proj=DT_PROJ, dt_sc=DT_SC, dt_av=DT_AV):
    key = (dt_proj, dt_sc, dt_av)
    if key not in _NC_CACHE:
        nc = bacc.Bacc("TRN2", target_bir_lowering=False, debug=False)
        with tile.TileContext(nc) as tc:
            nc._tc = tc
            _build(nc, n=N, d=D, hl=HL,
                   dt_proj=dt_proj, dt_sc=dt_sc, dt_av=dt_av)
        nc.compile()
        _NC_CACHE[key] = nc
    return _NC_CACHE[key]


def _in_maps(x, Wq, Wk, Wv):
    maps = []
    xT = [np.ascontiguousarray(x[b].T) for b in range(B)]
    WqT, WkT, WvT = Wq.T, Wk.T, Wv.T
    for c in range(N_CORES):
        bc, g = divmod(c, N_CORES // B)
        sl = slice(g * DL, (g + 1) * DL)
        maps.append({
            "xt": xT[bc],
            "wq": np.ascontiguousarray(WqT[:, sl]),
            "wk": np.ascontiguousarray(WkT[:, sl]),
            "wv": np.ascontiguousarray(WvT[:, sl]),
        })
    return maps


def kernel(x, Wq, Wk, Wv, _trace=False, _tmpdir=None, **_kw):
    x = np.asarray(x, dtype=np.float32)
    Wq = np.asarray(Wq, dtype=np.float32)
    Wk = np.asarray(Wk, dtype=np.float32)
    Wv = np.asarray(Wv, dtype=np.float32)
    assert x.shape == (B, N, D) and Wq.shape == (D, D)

    nc = _get_module()
    res = bass_utils.run_bass_kernel_spmd(
        nc, _in_maps(x, Wq, Wk, Wv), core_ids=list(range(N_CORES)),
        trace=_trace, tmpdir=_tmpdir)
    out = np.empty((B, N, D), np.float32)
    for c in range(N_CORES):
        bc, g = divmod(c, N_CORES // B)
        out[bc, :, g * DL:(g + 1) * DL] = res.results[c]["o"]
    if _trace:
        return out, res
    return out


# revision 6
# speedup vs baseline: 1.5810x; 1.5770x over previous
"""Causal multi-head attention (b=2, n=2048, d=768, 12 heads) on 8 TRN2 NeuronCores.

Sharding: batch x head-group. Core c handles batch c//4 and heads 3*(c%4) .. 3*(c%4)+2.
Each core gets xT = x[b].T plus W.T column slices for its 3 heads, computes the
unnormalized attention output (transposed) plus softmax denominators; the host
divides, transposes, and concatenates slabs into the full [2, 2048, 768].

Per-core algorithm (everything transposed so softmax reductions ride on matmuls):
  qT/kT/vT = (W.T slice).T @ xT            TensorE, per 512-col span
  v_nat[j, m] = transpose(vT) + ones column -> stationary [128, 65] per j-tile
  per head, per 512-col i-span:
    sT[j, i] = kT_h[:, jtile].T @ qT[:, span]   (psum, causally skipped/sliced)
    p = exp(sT)  unshifted (max causal score ~66 fits fp32); diagonal 128-blocks
        multiplied by a 0/1 triangular mask
    av[0:65, span] += v_nat[jtile].T @ p    (row 64 accumulates sum(p) = denom)
  av -> DRAM; host computes (av[0:64]/av[64]).T per head.

Perf-critical TRN2 facts baked in (measured on hardware):
  - every matmul keeps contraction K=128 (zero-padded kT/qT2): K<128 streams leave
    the PE HAM clock gate at 1.2 GHz and stop back-to-back pipelining (~3x slower)
  - float32r inputs: 1.06 cyc/row pipelined, ~12-bit mantissa
  - f32r tiles can only be produced by DVE/ACT compute ops (not memset/HWDGE-DMA),
    so zero-fills of f32r tiles are DVE copies from an f32 zeros tile
"""
import sys

if "/opt/trn_rl_repo" not in sys.path:
    sys.path.insert(0, "/opt/trn_rl_repo")

from contextlib import ExitStack

import numpy as np

import concourse.bass as bass
import concourse.tile as tile
from concourse import bacc, mybir, bass_utils
from concourse.masks import make_identity, make_upper_triangular

F32 = mybir.dt.float32
F32R = mybir.dt.float32r

P = 128
SPAN = 512
HD = 64

B, N, D, NH = 2, 2048, 768, 12
HL = 3                       # heads per core
DL = HL * HD                 # 192
N_CORES = 8
KT = D // P                  # 6 contraction chunks
NS = N // SPAN               # 4 spans
NT = N // P                  # 16 j-tiles
CPS = SPAN // P              # 4 chunks per span

DT_PROJ = F32R
DT_SC = F32R
DT_AV = F32R


def _build(nc, tc, dt_proj, dt_sc, dt_av):
    xt = nc.dram_tensor("xt", [D, N], dt_proj, kind="ExternalInput").ap()
    wq = nc.dram_tensor("wq", [D, DL], dt_proj, kind="ExternalInput").ap()
    wk = nc.dram_tensor("wk", [D, DL], dt_proj, kind="ExternalInput").ap()
    wv = nc.dram_tensor("wv", [D, DL], dt_proj, kind="ExternalInput").ap()
    o = nc.dram_tensor("o", [HL * (HD + 1), N], F32, kind="ExternalOutput").ap()

    with ExitStack() as ctx:
        pool = lambda name, bufs, **kw: ctx.enter_context(
            tc.tile_pool(name=name, bufs=bufs, **kw))
        const_pool = pool("const", 1)
        xpool = pool("x", KT * NS)
        wpool = pool("w", 3 * KT)
        qk_pool = pool("qk", NS)      # qT01, qT2z, (vT01, vT2z share size)
        kz_pool = pool("kz", HL * NS)
        vnat_pool = pool("vnat", HL)
        ppool = pool("p", 3)
        osb_pool = pool("osb", 3)
        ps_proj = pool("ps_proj", 2, space="PSUM")
        ps_vtr = pool("ps_vtr", 2, space="PSUM")
        ps_sc = pool("ps_sc", 2, space="PSUM")
        ps_av = pool("ps_av", 2, space="PSUM")

        ident = const_pool.tile([P, P], F32)
        make_identity(nc, ident[:])
        trimask = const_pool.tile([P, P], F32)
        make_upper_triangular(nc, trimask[:], val=1.0, diag=True)
        ones16 = const_pool.tile([P, NT], F32)
        nc.gpsimd.memset(ones16[:], 1.0)
        zeros = const_pool.tile([P, SPAN], F32)
        nc.gpsimd.memset(zeros[:], 0.0)

        def zfill(ap):
            """Zero a region of a possibly-f32r tile (memset can't write f32r)."""
            if ap.dtype == F32:
                nc.gpsimd.memset(ap, 0.0)
            else:
                nc.vector.tensor_copy(ap, zeros[0:ap.shape[0], 0:ap.shape[1]])

        # ---- DMA inputs ----
        w_tiles = {}
        for name, wsrc in (("q", wq), ("k", wk), ("v", wv)):
            w_tiles[name] = []
            for kt in range(KT):
                t = wpool.tile([P, DL], dt_proj, tag="w")
                nc.sync.dma_start(t[:], wsrc[kt * P:(kt + 1) * P, :])
                w_tiles[name].append(t)

        x_tiles = {}
        for ns in range(NS):
            for kt in range(KT):
                t = xpool.tile([P, SPAN], dt_proj, tag="x")
                nc.sync.dma_start(
                    t[:], xt[kt * P:(kt + 1) * P, ns * SPAN:(ns + 1) * SPAN])
                x_tiles[(kt, ns)] = t

        # ---- padded q/k/v layout (zero-filled up front, off critical path) ----
        qT01 = [qk_pool.tile([P, SPAN], dt_sc, tag="q01", name=f"q01_{i}") for i in range(NS)]
        qT2z = [qk_pool.tile([P, SPAN], dt_sc, tag="q2z", name=f"q2z_{i}") for i in range(NS)]
        vT01 = [qk_pool.tile([P, SPAN], F32, tag="v01", name=f"v01_{i}") for i in range(NS)]
        vT2z = [qk_pool.tile([P, SPAN], F32, tag="v2z", name=f"v2z_{i}") for i in range(NS)]
        kTz = [[kz_pool.tile([P, SPAN], dt_sc, tag="kz", name=f"kz_{h}_{i}")
                for i in range(NS)] for h in range(HL)]
        for ns in range(NS):
            zfill(qT2z[ns][HD:P, :])
            zfill(vT2z[ns][HD:P, :])
            zfill(kTz[0][ns][HD:P, :])
            zfill(kTz[1][ns][0:HD, :])
            zfill(kTz[2][ns][HD:P, :])

        v_nat = []
        for h in range(HL):
            vt = vnat_pool.tile([P, NT * (HD + 1)], dt_av, tag="vnat")
            v3 = vt[:].rearrange("p (t c) -> p t c", c=HD + 1)
            if dt_av == F32:
                nc.gpsimd.memset(v3[:, :, HD], 1.0)
            else:
                nc.vector.tensor_copy(v3[:, :, HD], ones16[:])
            v_nat.append(vt)

        # ---- projections (K=128 chunks, M chunks 128+64) ----
        m_chunks = ((0, P), (P, HD))
        for ns in range(NS):
            for name in ("q", "k", "v"):
                for (moff, msz) in m_chunks:
                    pt = ps_proj.tile([msz, SPAN], F32, tag="ps_proj")
                    for kt in range(KT):
                        nc.tensor.matmul(
                            pt[:],
                            w_tiles[name][kt][:, moff:moff + msz],
                            x_tiles[(kt, ns)][:],
                            start=(kt == 0), stop=(kt == KT - 1))
                    if name == "q":
                        if moff == 0:
                            nc.vector.tensor_copy(qT01[ns][:], pt[:])
                        else:
                            nc.vector.tensor_copy(qT2z[ns][0:HD, :], pt[:])
                    elif name == "k":
                        if moff == 0:
                            nc.vector.tensor_copy(kTz[0][ns][0:HD, :],
                                                  pt[0:HD, :])
                            nc.vector.tensor_copy(kTz[1][ns][HD:P, :],
                                                  pt[HD:P, :])
                        else:
                            nc.vector.tensor_copy(kTz[2][ns][0:HD, :], pt[:])
                    else:
                        if moff == 0:
                            nc.vector.tensor_copy(vT01[ns][:], pt[:])
                        else:
                            nc.vector.tensor_copy(vT2z[ns][0:HD, :], pt[:])

            # v -> natural layout for this span's 4 j-tiles (K=128 transposes)
            for c in range(CPS):
                jt = ns * CPS + c
                col = jt * (HD + 1)
                tp = ps_vtr.tile([P, P], F32, tag="ps_vtr")
                nc.tensor.transpose(tp[:], vT01[ns][:, c * P:(c + 1) * P],
                                    ident[:])
                nc.vector.tensor_copy(v_nat[0][:, col:col + HD], tp[:, 0:HD])
                nc.vector.tensor_copy(v_nat[1][:, col:col + HD], tp[:, HD:P])
                tp2 = ps_vtr.tile([P, P], F32, tag="ps_vtr")
                nc.tensor.transpose(tp2[:], vT2z[ns][:, c * P:(c + 1) * P],
                                    ident[:])
                nc.vector.tensor_copy(v_nat[2][:, col:col + HD], tp2[:, 0:HD])

        # ---- attention ----
        for h in range(HL):
            q_of = qT01 if h < 2 else qT2z
            for s in range(NS):
                av = ps_av.tile([HD + 1, SPAN], F32, tag="ps_av")
                njt = CPS * s + CPS
                for jt in range(njt):
                    c_d = jt - CPS * s
                    n0 = max(c_d, 0) * P
                    ns_k, ck = jt // CPS, jt % CPS
                    sc = ps_sc.tile([P, SPAN], F32, tag="ps_sc")
                    nc.tensor.matmul(
                        sc[:, n0:SPAN],
                        kTz[h][ns_k][:, ck * P:(ck + 1) * P],
                        q_of[s][:, n0:SPAN],
                        start=True, stop=True)
                    p = ppool.tile([P, SPAN], dt_av, tag="p")
                    nc.scalar.activation(p[:, n0:SPAN], sc[:, n0:SPAN],
                                         mybir.ActivationFunctionType.Exp)
                    if c_d >= 0:
                        nc.vector.tensor_mul(
                            p[:, n0:n0 + P], p[:, n0:n0 + P], trimask[:])
                    nc.tensor.matmul(
                        av[:, n0:SPAN],
                        v_nat[h][:, jt * (HD + 1):(jt + 1) * (HD + 1)],
                        p[:, n0:SPAN],
                        start=(jt == 0), stop=(jt == njt - 1))
                ob = osb_pool.tile([HD + 1, SPAN], F32, tag="osb")
                nc.vector.tensor_copy(ob[:], av[:])
                nc.sync.dma_start(
                    o[h * (HD + 1):(h + 1) * (HD + 1),
                      s * SPAN:(s + 1) * SPAN], ob[:])


_NC_CACHE = {}


def _get_module(dt_proj=DT_PROJ, dt_sc=DT_SC, dt_av=DT_AV):
    key = (dt_proj, dt_sc, dt_av)
    if key not in _NC_CACHE:
        nc = bacc.Bacc("TRN2", target_bir_lowering=False, debug=False)
        with tile.TileContext(nc) as tc:
            _build(nc, tc, dt_proj, dt_sc, dt_av)
        nc.compile()
        _NC_CACHE[key] = nc
    return _NC_CACHE[key]


def _in_maps(x, Wq, Wk, Wv):
    maps = []
    xT = [np.ascontiguousarray(x[b].T) for b in range(B)]
    WqT, WkT, WvT = Wq.T, Wk.T, Wv.T
    for c in range(N_CORES):
        bc, g = divmod(c, N_CORES // B)
        sl = slice(g * DL, (g + 1) * DL)
        maps.append({
            "xt": xT[bc],
            "wq": np.ascontiguousarray(WqT[:, sl]),
            "wk": np.ascontiguousarray(WkT[:, sl]),
            "wv": np.ascontiguousarray(WvT[:, sl]),
        })
    return maps


def kernel(x, Wq, Wk, Wv, _trace=False, _tmpdir=None, **_kw):
    x = np.asarray(x, dtype=np.float32)
    Wq = np.asarray(Wq, dtype=np.float32)
    Wk = np.asarray(Wk, dtype=np.float32)
    Wv = np.asarray(Wv, dtype=np.float32)
    assert x.shape == (B, N, D) and Wq.shape == (D, D)

    nc = _get_module()
    res = bass_utils.run_bass_kernel_spmd(
        nc, _in_maps(x, Wq, Wk, Wv), core_ids=list(range(N_CORES)),
        trace=_trace, tmpdir=_tmpdir)
    out = np.empty((B, N, D), np.float32)
    for c in range(N_CORES):
        bc, g = divmod(c, N_CORES // B)
        oT = res.results[c]["o"].astype(np.float64)
        for h in range(HL):
            blk = oT[h * (HD + 1):h * (HD + 1) + HD, :]
            den = oT[h * (HD + 1) + HD, :]
            out[bc, :, g * DL + h * HD:g * DL + (h + 1) * HD] = \
                (blk / den).T.astype(np.float32)
    if _trace:
        return out, res
    return out


# revision 9
# speedup vs baseline: 1.8291x; 1.1569x over previous
"""Causal multi-head attention (b=2, n=2048, d=768, 12 heads) on 8 TRN2 NeuronCores.

Sharding: batch x head-group. Core c handles batch c//4 and heads 3*(c%4) .. 3*(c%4)+2.
Each core gets xT = x[b].T plus W.T column slices for its 3 heads, computes the
unnormalized attention output (transposed) plus softmax denominators; the host
divides, transposes, and concatenates slabs into the full [2, 2048, 768].

Per-core algorithm (everything transposed so softmax reductions ride on matmuls):
  qT/kT/vT = (W.T slice).T @ xT            TensorE, per 512-col span
  v_nat[j, m] = transpose(vT) + ones column -> stationary [128, 65] per j-tile
  per head, per 512-col i-span:
    sT[j, i] = kT_h[:, jtile].T @ qT[:, span]   (psum, causally skipped/sliced)
    p = exp(sT)  unshifted (max causal score ~66 fits fp32); diagonal 128-blocks
        multiplied by a 0/1 triangular mask
    av[0:65, span] += v_nat[jtile].T @ p    (row 64 accumulates sum(p) = denom)
  av -> DRAM; host computes (av[0:64]/av[64]).T per head.

Perf-critical TRN2 facts baked in (measured on hardware):
  - every matmul keeps contraction K=128 (zero-padded kT/qT2): K<128 streams leave
    the PE HAM clock gate at 1.2 GHz and stop back-to-back pipelining (~3x slower)
  - float32r inputs: 1.06 cyc/row pipelined, ~12-bit mantissa
  - f32r tiles can only be produced by DVE/ACT compute ops (not memset/HWDGE-DMA),
    so zero-fills of f32r tiles are DVE copies from an f32 zeros tile
"""
import sys

if "/opt/trn_rl_repo" not in sys.path:
    sys.path.insert(0, "/opt/trn_rl_repo")

from contextlib import ExitStack

import numpy as np

import concourse.bass as bass
import concourse.tile as tile
from concourse import bacc, mybir, bass_utils
from concourse.masks import make_identity, make_upper_triangular

F32 = mybir.dt.float32
F32R = mybir.dt.float32r

P = 128
SPAN = 512
HD = 64

B, N, D, NH = 2, 2048, 768, 12
HL = 3                       # heads per core
DL = HL * HD                 # 192
N_CORES = 8
KT = D // P                  # 6 contraction chunks
NS = N // SPAN               # 4 spans
NT = N // P                  # 16 j-tiles
CPS = SPAN // P              # 4 chunks per span

DT_PROJ = F32R
DT_SC = F32R
DT_AV = F32R


def _build(nc, tc, dt_proj, dt_sc, dt_av):
    xt = nc.dram_tensor("xt", [D, N], dt_proj, kind="ExternalInput").ap()
    wq = nc.dram_tensor("wq", [D, DL], dt_proj, kind="ExternalInput").ap()
    wk = nc.dram_tensor("wk", [D, DL], dt_proj, kind="ExternalInput").ap()
    wv = nc.dram_tensor("wv", [D, DL], dt_proj, kind="ExternalInput").ap()
    o = nc.dram_tensor("o", [HL * (HD + 1), N], F32, kind="ExternalOutput").ap()

    with ExitStack() as ctx:
        pool = lambda name, bufs, **kw: ctx.enter_context(
            tc.tile_pool(name=name, bufs=bufs, **kw))
        const_pool = pool("const", 1)
        xpool = pool("x", NS)
        wpool = pool("w", 3)
        qk_pool = pool("qk", NS)      # qT01, qT2z, (vT01, vT2z share size)
        kz_pool = pool("kz", HL * NS)
        vnat_pool = pool("vnat", HL)
        ppool = pool("p", 3)
        osb_pool = pool("osb", 3)
        ps_proj = pool("ps_proj", 2, space="PSUM")
        ps_vtr = pool("ps_vtr", 1, space="PSUM")
        ps_sc = pool("ps_sc", 2, space="PSUM")
        ps_av = pool("ps_av", 3, space="PSUM")

        ident = const_pool.tile([P, P], F32)
        make_identity(nc, ident[:])
        trimask = const_pool.tile([P, P], F32)
        make_upper_triangular(nc, trimask[:], val=1.0, diag=True)
        ones32 = const_pool.tile([P, 2 * NT], F32)
        nc.gpsimd.memset(ones32[:], 1.0)
        zeros = const_pool.tile([P, SPAN], F32)
        nc.gpsimd.memset(zeros[:], 0.0)

        def zfill(ap):
            """Zero a region of a possibly-f32r tile (memset can't write f32r)."""
            if ap.dtype == F32:
                nc.gpsimd.memset(ap, 0.0)
            else:
                nc.vector.tensor_copy(ap, zeros[0:ap.shape[0], 0:ap.shape[1]])

        # ---- DMA inputs (one batched 3D-AP DMA per tensor / span) ----
        w_tiles = {}
        for name, wsrc in (("q", wq), ("k", wk), ("v", wv)):
            t = wpool.tile([P, KT * DL], dt_proj, tag="w", name=f"w{name}")
            nc.sync.dma_start(
                t[:].rearrange("p (k m) -> p k m", m=DL),
                wsrc.rearrange("(k p) m -> p k m", p=P))
            w_tiles[name] = t

        x_tiles = []
        for ns in range(NS):
            t = xpool.tile([P, KT * SPAN], dt_proj, tag="x", name=f"x{ns}")
            nc.sync.dma_start(
                t[:].rearrange("p (k c) -> p k c", c=SPAN),
                xt.rearrange("(k p) n -> p k n", p=P)[
                    :, :, ns * SPAN:(ns + 1) * SPAN])
            x_tiles.append(t)

        # ---- padded q/k/v layout (zero-filled up front, off critical path) ----
        qT01 = [qk_pool.tile([P, SPAN], dt_sc, tag="q01", name=f"q01_{i}") for i in range(NS)]
        qT2z = [qk_pool.tile([P, SPAN], dt_sc, tag="q2z", name=f"q2z_{i}") for i in range(NS)]
        vT01 = [qk_pool.tile([P, SPAN], F32, tag="v01", name=f"v01_{i}") for i in range(NS)]
        vT2z = [qk_pool.tile([P, SPAN], F32, tag="v2z", name=f"v2z_{i}") for i in range(NS)]
        kTz = [[kz_pool.tile([P, SPAN], dt_sc, tag="kz", name=f"kz_{h}_{i}")
                for i in range(NS)] for h in range(HL)]
        for ns in range(NS):
            zfill(qT2z[ns][HD:P, :])
            zfill(vT2z[ns][HD:P, :])
            zfill(kTz[0][ns][HD:P, :])
            zfill(kTz[1][ns][0:HD, :])
            zfill(kTz[2][ns][HD:P, :])

        # v in natural layout: heads 0,1 interleaved per j-tile as
        # [v0(64) | 1 | v1(64) | 1]; head 2 separate as [v2(64) | 1].
        v_nat01 = vnat_pool.tile([P, NT * 2 * (HD + 1)], dt_av, tag="vnat01")
        v_nat2 = vnat_pool.tile([P, NT * (HD + 1)], dt_av, tag="vnat2")
        c01 = v_nat01[:].rearrange("p (t c) -> p t c", c=HD + 1)[:, :, HD]
        c2 = v_nat2[:].rearrange("p (t c) -> p t c", c=HD + 1)[:, :, HD]
        if dt_av == F32:
            nc.gpsimd.memset(c01, 1.0)
            nc.gpsimd.memset(c2, 1.0)
        else:
            nc.vector.tensor_copy(c01, ones32[:])
            nc.vector.tensor_copy(c2, ones32[:, 0:NT])

        def vnat_lhsT(h, jt):
            if h < 2:
                b = jt * 2 * (HD + 1) + h * (HD + 1)
                return v_nat01[:, b:b + HD + 1]
            b = jt * (HD + 1)
            return v_nat2[:, b:b + HD + 1]

        # ---- projections (K=128 chunks, M chunks 128+64) ----
        m_chunks = ((0, P), (P, HD))
        for ns in range(NS):
            for name in ("q", "k", "v"):
                for (moff, msz) in m_chunks:
                    pt = ps_proj.tile([msz, SPAN], F32, tag="ps_proj")
                    for kt in range(KT):
                        nc.tensor.matmul(
                            pt[:],
                            w_tiles[name][:, kt * DL + moff:kt * DL + moff + msz],
                            x_tiles[ns][:, kt * SPAN:(kt + 1) * SPAN],
                            start=(kt == 0), stop=(kt == KT - 1))
                    if name == "q":
                        if moff == 0:
                            nc.vector.tensor_copy(qT01[ns][:], pt[:])
                        else:
                            nc.vector.tensor_copy(qT2z[ns][0:HD, :], pt[:])
                    elif name == "k":
                        if moff == 0:
                            nc.vector.tensor_copy(kTz[0][ns][0:HD, :],
                                                  pt[0:HD, :])
                            nc.vector.tensor_copy(kTz[1][ns][HD:P, :],
                                                  pt[HD:P, :])
                        else:
                            nc.vector.tensor_copy(kTz[2][ns][0:HD, :], pt[:])
                    else:
                        if moff == 0:
                            nc.vector.tensor_copy(vT01[ns][:], pt[:])
                        else:
                            nc.vector.tensor_copy(vT2z[ns][0:HD, :], pt[:])

            # v -> natural layout for this span's 4 j-tiles (K=128 transposes)
            for c in range(CPS):
                jt = ns * CPS + c
                tp = ps_vtr.tile([P, P], F32, tag="ps_vtr")
                nc.tensor.transpose(tp[:], vT01[ns][:, c * P:(c + 1) * P],
                                    ident[:])
                b01 = jt * 2 * (HD + 1)
                nc.vector.tensor_copy(
                    v_nat01[:].rearrange("p (t c) -> p t c", c=HD + 1)[
                        :, 2 * jt:2 * jt + 2, 0:HD],
                    tp[:].rearrange("p (t c) -> p t c", c=HD))
                tp2 = ps_vtr.tile([P, P], F32, tag="ps_vtr")
                nc.tensor.transpose(tp2[:], vT2z[ns][:, c * P:(c + 1) * P],
                                    ident[:])
                nc.vector.tensor_copy(
                    v_nat2[:, jt * (HD + 1):jt * (HD + 1) + HD], tp2[:, 0:HD])

            # ---- attention: span s == ns only needs projections <= ns ----
            s = ns
            for h in range(HL):
                q_of = qT01 if h < 2 else qT2z
                av = ps_av.tile([HD + 1, SPAN], F32, tag="ps_av")
                njt = CPS * s + CPS
                for jt in range(njt):
                    c_d = jt - CPS * s
                    n0 = max(c_d, 0) * P
                    ns_k, ck = jt // CPS, jt % CPS
                    sc = ps_sc.tile([P, SPAN], F32, tag="ps_sc")
                    nc.tensor.matmul(
                        sc[:, n0:SPAN],
                        kTz[h][ns_k][:, ck * P:(ck + 1) * P],
                        q_of[s][:, n0:SPAN],
                        start=True, stop=True)
                    p = ppool.tile([P, SPAN], dt_av, tag="p")
                    nc.scalar.activation(p[:, n0:SPAN], sc[:, n0:SPAN],
                                         mybir.ActivationFunctionType.Exp)
                    if c_d >= 0:
                        nc.vector.tensor_mul(
                            p[:, n0:n0 + P], p[:, n0:n0 + P], trimask[:])
                    nc.tensor.matmul(
                        av[:, n0:SPAN],
                        vnat_lhsT(h, jt),
                        p[:, n0:SPAN],
                        start=(jt == 0), stop=(jt == njt - 1))
                ob = osb_pool.tile([HD + 1, SPAN], F32, tag="osb")
                nc.vector.tensor_copy(ob[:], av[:])
                nc.sync.dma_start(
                    o[h * (HD + 1):(h + 1) * (HD + 1),
                      s * SPAN:(s + 1) * SPAN], ob[:])


_NC_CACHE = {}


def _get_module(dt_proj=DT_PROJ, dt_sc=DT_SC, dt_av=DT_AV):
    key = (dt_proj, dt_sc, dt_av)
    if key not in _NC_CACHE:
        nc = bacc.Bacc("TRN2", target_bir_lowering=False, debug=False)
        with tile.TileContext(nc) as tc:
            _build(nc, tc, dt_proj, dt_sc, dt_av)
        nc.compile()
        _NC_CACHE[key] = nc
    return _NC_CACHE[key]


def _in_maps(x, Wq, Wk, Wv):
    maps = []
    xT = [np.ascontiguousarray(x[b].T) for b in range(B)]
    WqT, WkT, WvT = Wq.T, Wk.T, Wv.T
    for c in range(N_CORES):
        bc, g = divmod(c, N_CORES // B)
        sl = slice(g * DL, (g + 1) * DL)
        maps.append({
            "xt": xT[bc],
            "wq": np.ascontiguousarray(WqT[:, sl]),
            "wk": np.ascontiguousarray(WkT[:, sl]),
            "wv": np.ascontiguousarray(WvT[:, sl]),
        })
    return maps


def kernel(x, Wq, Wk, Wv, _trace=False, _tmpdir=None, **_kw):
    x = np.asarray(x, dtype=np.float32)
    Wq = np.asarray(Wq, dtype=np.float32)
    Wk = np.asarray(Wk, dtype=np.float32)
    Wv = np.asarray(Wv, dtype=np.float32)
    assert x.shape == (B, N, D) and Wq.shape == (D, D)

    nc = _get_module()
    res = bass_utils.run_bass_kernel_spmd(
        nc, _in_maps(x, Wq, Wk, Wv), core_ids=list(range(N_CORES)),
        trace=_trace, tmpdir=_tmpdir)
    out = np.empty((B, N, D), np.float32)
    for c in range(N_CORES):
        bc, g = divmod(c, N_CORES // B)
        oT = res.results[c]["o"].astype(np.float64)
        for h in range(HL):
            blk = oT[h * (HD + 1):h * (HD + 1) + HD, :]
            den = oT[h * (HD + 1) + HD, :]
            out[bc, :, g * DL + h * HD:g * DL + (h + 1) * HD] = \
                (blk / den).T.astype(np.float32)
    if _trace:
        return out, res
    return out
